# revision 1
# baseline (speedup 1.0000x reference)
"""Chamfer distance kernel for Trainium2 (8 NeuronCores, Bass/Tile).

Problem: cloud1, cloud2: (4, 8192, 3) f32.  For each batch n:
  out[n] = mean_p min_q ||c1[p]-c2[q]||^2 + mean_q min_p ||c2[q]-c1[p]||^2

Strategy (one batch-direction per core; 4 batches x 2 directions = 8 cores):
  min_q ||a_p - b_q||^2 = 2*(|a_p|^2/2 - max_q (a_p . b_q - |b_q|^2/2))

The whole per-pair term (a_p . b_q - |b_q|^2/2) is produced by ONE bf16
matmul with an augmented K=30 contraction:
  - each coordinate of both clouds is split into 3 bf16 terms
    (hi/mid/lo, x = h+m+l exactly to ~2^-24), and all 9 cross products
    per coordinate are rows of the contraction -> fp32-grade a.b,
  - 3 more rows pair a ones-row with the 3-term bf16 split of -|b|^2/2.
bf16 streams through the PE at 1 row/cycle (fp32 would be 4x slower).

The 8192x8192 score matrix per core is never materialized: it is
produced 128x1024 at a time into PSUM and consumed by an online max.
The max-reduce is the bottleneck (DVE tensor_reduce is 1 elem/cycle/lane)
so we register a custom DVE op (TTR_MAX_ANT, see _register_ttr_max):
each DVE pass consumes TWO 1024-wide chunks (one straight from PSUM, one
staged PSUM->SBUF by the otherwise-idle scalar engine), halving DVE time.
Measured ~355-375 us per core per full batch-direction on HW.

Host-side (numpy) work is only layout prep + O(P) reductions:
coordinate transposes/splits, |a|^2, |b|^2, and the final combine of the
eight per-core [128] partial sums.
"""

import functools
from contextlib import ExitStack, nullcontext

import numpy as np
import ml_dtypes

try:
    import concourse.bass as bass
except ImportError:  # fallback if the site path isn't preconfigured
    import sys

    sys.path.insert(0, "/opt/trn_rl_repo")
    import concourse.bass as bass

import jax
import concourse.tile as tile
import concourse.dve_ops as dve_ops
from concourse import bacc, mybir
from concourse import bass2jax
from concourse.dve_spec import Spec, Src0, Src1, C0, maxx, lower as dve_lower
from concourse.dve_uop import DveOpSpec
from jax.sharding import Mesh, PartitionSpec
from jax.experimental.shard_map import shard_map

P_PTS = 8192
N_CORES = 8
K_ROWS = 30
CHUNK = 1024  # q-chunk width = 2 PSUM banks
SCHEME = "ttr"  # "ttr" (ACT-staged dual-stream max) or "simple" (DVE reduce)
NEG_INF = -3.0e38

BF16 = ml_dtypes.bfloat16


# ----------------------------------------------------------------- host prep


def _split3(x):
    """3-term bf16 split: parts sum to x with ~2^-24 relative error."""
    x = np.asarray(x, np.float64)
    h = x.astype(BF16)
    r = x - h.astype(np.float64)
    m = r.astype(BF16)
    l = (r - m.astype(np.float64)).astype(BF16)
    return h, m, l


def _prep_side(A, B):
    """Build per-core inputs for direction 'for each point of A, min over B'.

    Returns (lhs [K,P] bf16, rhs [K,P] bf16, sum_half_a2 float).
    Device computes S = sum_p max_q sum_k lhs[k,p]*rhs[k,q]; then
    mean_p min_q ||a_p-b_q||^2 = 2*(sum_half_a2 - S)/P.
    """
    P = A.shape[0]
    ka, kb = [], []
    for d in range(3):
        ah, am, al = _split3(A[:, d])
        bh, bm, bl = _split3(B[:, d])
        for ap in (ah, am, al):
            for bp in (bh, bm, bl):
                ka.append(ap)
                kb.append(bp)
    b2h = 0.5 * np.sum(np.asarray(B, np.float64) ** 2, axis=1)
    ones = np.ones(P, BF16)
    for part in _split3(b2h):
        ka.append(ones)
        kb.append((-part.astype(np.float64)).astype(BF16))
    lhs = np.stack(ka).astype(BF16)
    rhs = np.stack(kb).astype(BF16)
    assert lhs.shape == (K_ROWS, P) and rhs.shape == (K_ROWS, P)
    sum_half_a2 = 0.5 * float(np.sum(np.asarray(A, np.float64) ** 2))
    return lhs, rhs, sum_half_a2


# --------------------------------------------------- custom DVE op (TTR max)
#
# Stock nc.vector.tensor_tensor_reduce LOOKS generic (op0/op1 fields), but
# the hardware uop table only implements the mult/add (dot-product) variant;
# a max/max call executes undefined firmware and crashes the exec unit.
# The custom-DVE framework is the sanctioned way to add new fused DVE ops:
# we register a dual-stream max:
#   out[k] = max(in0[k], in1[k]);  accum_out = max(s0, max_k out[k])
# One DVE pass consumes TWO chunks (2 elems/cycle total) and chains via s0.


def _register_ttr_max():
    name = "TTR_MAX_ANT"
    for o in dve_ops.OPS:
        if o.name == name:
            return o
    def _ref(in0, in1, c0, c1, c2):
        body = np.maximum(in0.astype(np.float32), in1.astype(np.float32))
        seed = np.asarray(c0, np.float32).reshape(-1, 1)
        return body, np.maximum(body.max(axis=-1, keepdims=True), seed)

    spec = Spec(body=maxx(Src0, Src1), accum=maxx, accum_init=C0, reference=_ref)
    row = dve_ops._CUSTOM_DVE_ROW_BASE + len(dve_ops.OPS)
    shas = {}
    for ver in ("v3", "v4"):
        uops = dve_lower(spec, ver=ver)
        shas[ver] = DveOpSpec(
            name=name, opcode=row, uops=uops, rd1_en=True
        ).sha(ver)
    op = dve_ops.DveOp(name, spec, subdim=False, uops_sha=shas)
    dve_ops.OPS.append(op)
    dve_ops._SUB_OPCODE_FOR_NAME[name] = row
    dve_ops.CUSTOM_DVE_SPECS[name] = op.spec
    return op


TTR_MAX = _register_ttr_max()


# ------------------------------------------------------------- device kernel


def _emit(nc, scheme, p_pts, chunk, reps):
    f32 = mybir.dt.float32
    bf16 = mybir.dt.bfloat16
    X = mybir.AxisListType.X
    MAX = mybir.AluOpType.max

    lhs_d = nc.dram_tensor("lhs", [K_ROWS, p_pts], bf16, kind="ExternalInput").ap()
    rhs_d = nc.dram_tensor("rhs", [K_ROWS, p_pts], bf16, kind="ExternalInput").ap()
    out_d = nc.dram_tensor("out", [128, 1], f32, kind="ExternalOutput").ap()

    nb = p_pts // 128  # p-blocks
    nch = p_pts // chunk  # q-chunks per block
    mmpc = chunk // 512  # matmuls per chunk

    with tile.TileContext(nc) as tc, ExitStack() as ctx:
        inp = ctx.enter_context(tc.tile_pool(name="inp", bufs=1))
        psump = ctx.enter_context(
            tc.tile_pool(name="psum", bufs=4, space=bass.MemorySpace.PSUM)
        )
        stagep = ctx.enter_context(tc.tile_pool(name="stage", bufs=3))
        junkp = ctx.enter_context(tc.tile_pool(name="junk", bufs=3))
        accp = ctx.enter_context(tc.tile_pool(name="acc", bufs=6))
        resp = ctx.enter_context(tc.tile_pool(name="res", bufs=1))

        lhs_sb = inp.tile([K_ROWS, p_pts], bf16, tag="lhs")
        rhs_sb = inp.tile([K_ROWS, p_pts], bf16, tag="rhs")
        nc.sync.dma_start(lhs_sb[:], lhs_d[:])
        nc.sync.dma_start(rhs_sb[:], rhs_d[:])

        # reps>1: hardware loop around the body — used only for timing
        # (amortizes host/RPC overhead without growing code size)
        loop_cm = tc.For_i(0, reps, 1) if reps > 1 else nullcontext()
        with loop_cm:
            blockmax = resp.tile([128, nb], f32, tag="blockmax")
            if scheme == "simple":
                chmax = resp.tile([128, nb * nch], f32, tag="chmax")
            elif scheme == "ttr2":
                # unchained: one column per chunk-pair, tree-reduced at end
                chmax = resp.tile([128, nb * (nch // 2)], f32, tag="chmax")

            for i in range(nb):
                wt = lhs_sb[:, i * 128 : (i + 1) * 128]

                if scheme == "simple":
                    for j in range(nch):
                        ps = psump.tile([128, chunk], f32, tag="ps")
                        for m in range(mmpc):
                            q0 = j * chunk + m * 512
                            nc.tensor.matmul(
                                ps[:, m * 512 : (m + 1) * 512],
                                wt,
                                rhs_sb[:, q0 : q0 + 512],
                                start=True,
                                stop=True,
                            )
                        col = i * nch + j
                        nc.vector.reduce_max(
                            chmax[:, col : col + 1], ps[:], axis=X
                        )
                else:  # "ttr": pairs of chunks, ACT stages the odd one
                    acc_ap = None
                    for j in range(0, nch, 2):
                        psA = psump.tile([128, chunk], f32, tag="ps")
                        for m in range(mmpc):
                            q0 = j * chunk + m * 512
                            nc.tensor.matmul(
                                psA[:, m * 512 : (m + 1) * 512],
                                wt,
                                rhs_sb[:, q0 : q0 + 512],
                                start=True,
                                stop=True,
                            )
                        psB = psump.tile([128, chunk], f32, tag="ps")
                        for m in range(mmpc):
                            q0 = (j + 1) * chunk + m * 512
                            nc.tensor.matmul(
                                psB[:, m * 512 : (m + 1) * 512],
                                wt,
                                rhs_sb[:, q0 : q0 + 512],
                                start=True,
                                stop=True,
                            )
                        st = stagep.tile([128, chunk], f32, tag="st")
                        nc.scalar.copy(st[:], psB[:])
                        junk = junkp.tile([128, chunk], f32, tag="junk")
                        if scheme == "ttr2":
                            col = i * (nch // 2) + j // 2
                            accout = chmax[:, col : col + 1]
                            seed = NEG_INF
                        else:
                            last = j + 2 >= nch
                            if last:
                                accout = blockmax[:, i : i + 1]
                            else:
                                accout = accp.tile(
                                    [128, 1], f32, name="acc", tag="acc"
                                )[:]
                            seed = NEG_INF if acc_ap is None else acc_ap
                        nc.vector._custom_dve(
                            TTR_MAX,
                            out=junk[:],
                            in0=psA[:],
                            in1=st[:],
                            s0=seed,
                            accum_out=accout,
                        )
                        acc_ap = accout

            if scheme == "simple":
                v = chmax[:].rearrange("p (b c) -> p b c", c=nch)
                nc.vector.tensor_reduce(blockmax[:], v, axis=X, op=MAX)
            elif scheme == "ttr2":
                v = chmax[:].rearrange("p (b c) -> p b c", c=nch // 2)
                nc.vector.tensor_reduce(blockmax[:], v, axis=X, op=MAX)

            sums = resp.tile([128, 1], f32, tag="sums")
            nc.vector.reduce_sum(sums[:], blockmax[:], axis=X)
            nc.sync.dma_start(out_d[:], sums[:])


@functools.lru_cache(maxsize=4)
def _build(scheme="ttr", p_pts=P_PTS, chunk=CHUNK, reps=1):
    nc = bacc.Bacc(
        "TRN2", target_bir_lowering=False, debug=False, num_devices=N_CORES
    )
    _emit(nc, scheme, p_pts, chunk, reps)
    nc.compile()
    return nc


# ---------------------------------------------------------------- executor


class _Exec:
    """Cached jitted SPMD executable for a built Bass module (axon/PJRT)."""

    def __init__(self, nc, n_cores=N_CORES):
        bass2jax.install_neuronx_cc_hook()
        self.nc = nc
        self.n_cores = n_cores
        partition_name = (
            nc.partition_id_tensor.name if nc.partition_id_tensor else None
        )
        in_names, out_names, out_avals = [], [], []
        for alloc in nc.m.functions[0].allocations:
            if not isinstance(alloc, mybir.MemoryLocationSet):
                continue
            name = alloc.memorylocations[0].name
            if alloc.kind == "ExternalInput":
                if name != partition_name:
                    in_names.append(name)
            elif alloc.kind == "ExternalOutput":
                out_names.append(name)
                out_avals.append(
                    jax.core.ShapedArray(
                        tuple(alloc.tensor_shape), mybir.dt.np(alloc.dtype)
                    )
                )
        self.in_names = in_names
        self.out_names = out_names
        self.out_avals = out_avals
        n_params = len(in_names)
        all_names = list(in_names + out_names)
        if partition_name is not None:
            all_names.append(partition_name)
        donate = tuple(range(n_params, n_params + len(out_names)))

        def _body(*args):
            operands = list(args)
            if partition_name is not None:
                operands.append(bass2jax.partition_id_tensor())
            return tuple(
                bass2jax._bass_exec_p.bind(
                    *operands,
                    out_avals=tuple(out_avals),
                    in_names=tuple(all_names),
                    out_names=tuple(out_names),
                    lowering_input_output_aliases=(),
                    sim_require_finite=True,
                    sim_require_nnan=True,
                    nc=nc,
                )
            )

        devices = jax.devices()[:n_cores]
        assert len(devices) == n_cores
        mesh = Mesh(np.asarray(devices), ("core",))
        specs = (PartitionSpec("core"),) * (n_params + len(out_names))
        self._fn = jax.jit(
            shard_map(
                _body,
                mesh=mesh,
                in_specs=specs,
                out_specs=(PartitionSpec("core"),) * len(out_names),
                check_rep=False,
            ),
            donate_argnums=donate,
            keep_unused=True,
        )

    def _concat_inputs(self, in_maps):
        return [
            np.concatenate([np.asarray(m[name]) for m in in_maps], axis=0)
            for name in self.in_names
        ]

    def _zeros(self):
        return [
            np.zeros((self.n_cores * a.shape[0], *a.shape[1:]), a.dtype)
            for a in self.out_avals
        ]

    def run(self, in_maps):
        outs = self._fn(*self._concat_inputs(in_maps), *self._zeros())
        return [
            {
                name: np.asarray(outs[i]).reshape(
                    self.n_cores, *self.out_avals[i].shape
                )[c]
                for i, name in enumerate(self.out_names)
            }
            for c in range(self.n_cores)
        ]

    def time(self, in_maps, iters=20, repeats=3):
        """Per-call wall time (s), inputs device-resident, min over repeats."""
        import time as _time

        cin = [jax.device_put(x) for x in self._concat_inputs(in_maps)]
        jax.block_until_ready(cin)
        outs = self._fn(*cin, *self._zeros())  # warm
        jax.block_until_ready(outs)
        best = float("inf")
        for _ in range(repeats):
            t0 = _time.perf_counter()
            last = None
            for _ in range(iters):
                last = self._fn(*cin, *self._zeros())
            jax.block_until_ready(last)
            t1 = _time.perf_counter()
            best = min(best, (t1 - t0) / iters)
        return best


@functools.lru_cache(maxsize=4)
def _get_exec(scheme="ttr", p_pts=P_PTS, chunk=CHUNK, reps=1):
    return _Exec(_build(scheme, p_pts, chunk, reps))


# ------------------------------------------------------------------- kernel


def _make_in_maps(cloud1, cloud2):
    cloud1 = np.asarray(cloud1)
    cloud2 = np.asarray(cloud2)
    n_batch = cloud1.shape[0]
    assert n_batch * 2 == N_CORES
    in_maps, halves = [], []
    for n in range(n_batch):
        for A, B in ((cloud1[n], cloud2[n]), (cloud2[n], cloud1[n])):
            lhs, rhs, sum_half_a2 = _prep_side(A, B)
            in_maps.append({"lhs": lhs, "rhs": rhs})
            halves.append(sum_half_a2)
    return in_maps, halves


def _combine(results, halves, n_batch, p_pts=P_PTS):
    out = np.zeros(n_batch, np.float64)
    for c in range(len(results)):
        S = float(np.asarray(results[c]["out"], np.float64).sum())
        out[c // 2] += 2.0 * (halves[c] - S) / p_pts
    return out.astype(np.float32)


def kernel(cloud1, cloud2):
    in_maps, halves = _make_in_maps(cloud1, cloud2)
    ex = _get_exec(SCHEME, P_PTS, CHUNK, 1)
    results = ex.run(in_maps)
    return _combine(results, halves, np.asarray(cloud1).shape[0])



# revision 21
# speedup vs baseline: 1.8574x; 1.8574x over previous
"""Chamfer distance kernel for Trainium2 (8 NeuronCores, Bass/Tile).

Problem: cloud1, cloud2: (4, 8192, 3) f32.  For each batch n:
  out[n] = mean_p min_q ||c1[p]-c2[q]||^2 + mean_q min_p ||c2[q]-c1[p]||^2

Strategy (one batch-direction per core; 4 batches x 2 directions = 8 cores):
  min_q ||a_p - b_q||^2 = 2*(|a_p|^2/2 - max_q (a_p . b_q - |b_q|^2/2))

The whole per-pair term (a_p . b_q - |b_q|^2/2) is produced by ONE bf16
matmul with an augmented K=30 contraction:
  - each coordinate of both clouds is split into 3 bf16 terms
    (hi/mid/lo, x = h+m+l exactly to ~2^-24), and all 9 cross products
    per coordinate are rows of the contraction -> fp32-grade a.b,
  - 3 more rows pair a ones-row with the 3-term bf16 split of -|b|^2/2.
bf16 streams through the PE at 1 row/cycle (fp32 would be 4x slower).

The 8192x8192 score matrix per core is never materialized: it is
produced 128x512 at a time into PSUM and consumed by an online max.
The max-reduce is the bottleneck (DVE tensor_reduce is 1 elem/cycle/lane)
so we register a custom DVE op (TTR_MAX_ANT, see _register_ttr_max):
each DVE pass consumes TWO 512-wide chunks (one straight from PSUM, one
staged PSUM->SBUF by the otherwise-idle scalar engine), halving DVE time.

Tuned configuration (scheme "ttr3", chunk 512, measured by reps-slope):
  - ACT stages the FIRST-produced chunk of each pair, so the scalar
    engine has a full pipeline slot of lead time and the DVE never
    waits on it; the DVE reads the second chunk live from PSUM.
  - accumulators are unchained (one chmax column per pair, tree-reduced
    at the end) to remove the serial s0-seed dependency between ops.
  - chunk=512 -> 8 single-bank PSUM tiles -> 4 pairs in flight; pool
    depth 3 for stage/junk is a sharp optimum (2/4/8 are 15-60%% worse).
Scheme "rt" additionally places consecutive p-blocks in alternating
PE row-groups (32*(i%2), via SBUF base-partition placement; K=30 <= 32)
so adjacent blocks' matmuls run concurrently in independent 32-row
subarrays - the PE stops pacing the DVE chain at block boundaries.
Progression measured here (reps-slope, all 8 cores in parallel):
432 us (ttr@1024) -> 393 (ttr2@1024) -> 259 (ttr3@1024) ->
243 (ttr3@512) -> 226 us (rt@512).

Host-side (numpy) work is only layout prep + O(P) reductions:
coordinate transposes/splits, |a|^2, |b|^2, and the final combine of the
eight per-core [128] partial sums.
"""

import functools
from contextlib import ExitStack, nullcontext

import numpy as np
import ml_dtypes

try:
    import concourse.bass as bass
except ImportError:  # fallback if the site path isn't preconfigured
    import sys

    sys.path.insert(0, "/opt/trn_rl_repo")
    import concourse.bass as bass

import jax
import concourse.tile as tile
import concourse.dve_ops as dve_ops
from concourse import bacc, mybir
from concourse import bass2jax
from concourse.dve_spec import Spec, Src0, Src1, C0, maxx, lower as dve_lower
from concourse.dve_uop import DveOpSpec
from jax.sharding import Mesh, PartitionSpec
from jax.experimental.shard_map import shard_map

P_PTS = 8192
N_CORES = 8
K_ROWS = 30
CHUNK = 512  # q-chunk width = 1 PSUM bank
SCHEME = "rt"  # row-tiled ttr3: 2-way PE row-groups + stage-first dual-stream max
NEG_INF = -3.0e38

BF16 = ml_dtypes.bfloat16


# ----------------------------------------------------------------- host prep


def _split3(x):
    """3-term bf16 split: parts sum to x with ~2^-24 relative error."""
    x = np.asarray(x, np.float64)
    h = x.astype(BF16)
    r = x - h.astype(np.float64)
    m = r.astype(BF16)
    l = (r - m.astype(np.float64)).astype(BF16)
    return h, m, l


def _prep_side(A, B):
    """Build per-core inputs for direction 'for each point of A, min over B'.

    Returns (lhs [K,P] bf16, rhs [K,P] bf16, sum_half_a2 float).
    Device computes S = sum_p max_q sum_k lhs[k,p]*rhs[k,q]; then
    mean_p min_q ||a_p-b_q||^2 = 2*(sum_half_a2 - S)/P.
    """
    P = A.shape[0]
    ka, kb = [], []
    for d in range(3):
        ah, am, al = _split3(A[:, d])
        bh, bm, bl = _split3(B[:, d])
        for ap in (ah, am, al):
            for bp in (bh, bm, bl):
                ka.append(ap)
                kb.append(bp)
    b2h = 0.5 * np.sum(np.asarray(B, np.float64) ** 2, axis=1)
    ones = np.ones(P, BF16)
    for part in _split3(b2h):
        ka.append(ones)
        kb.append((-part.astype(np.float64)).astype(BF16))
    lhs = np.stack(ka).astype(BF16)
    rhs = np.stack(kb).astype(BF16)
    assert lhs.shape == (K_ROWS, P) and rhs.shape == (K_ROWS, P)
    sum_half_a2 = 0.5 * float(np.sum(np.asarray(A, np.float64) ** 2))
    return lhs, rhs, sum_half_a2


# --------------------------------------------------- custom DVE op (TTR max)
#
# Stock nc.vector.tensor_tensor_reduce LOOKS generic (op0/op1 fields), but
# the hardware uop table only implements the mult/add (dot-product) variant;
# a max/max call executes undefined firmware and crashes the exec unit.
# The custom-DVE framework is the sanctioned way to add new fused DVE ops:
# we register a dual-stream max:
#   out[k] = max(in0[k], in1[k]);  accum_out = max(s0, max_k out[k])
# One DVE pass consumes TWO chunks (2 elems/cycle total) and chains via s0.


def _register_ttr_max():
    name = "TTR_MAX_ANT"
    for o in dve_ops.OPS:
        if o.name == name:
            return o
    def _ref(in0, in1, c0, c1, c2):
        body = np.maximum(in0.astype(np.float32), in1.astype(np.float32))
        seed = np.asarray(c0, np.float32).reshape(-1, 1)
        return body, np.maximum(body.max(axis=-1, keepdims=True), seed)

    spec = Spec(body=maxx(Src0, Src1), accum=maxx, accum_init=C0, reference=_ref)
    row = dve_ops._CUSTOM_DVE_ROW_BASE + len(dve_ops.OPS)
    shas = {}
    for ver in ("v3", "v4"):
        uops = dve_lower(spec, ver=ver)
        shas[ver] = DveOpSpec(
            name=name, opcode=row, uops=uops, rd1_en=True
        ).sha(ver)
    op = dve_ops.DveOp(name, spec, subdim=False, uops_sha=shas)
    dve_ops.OPS.append(op)
    dve_ops._SUB_OPCODE_FOR_NAME[name] = row
    dve_ops.CUSTOM_DVE_SPECS[name] = op.spec
    return op


TTR_MAX = _register_ttr_max()


# ------------------------------------------------------------- device kernel


def _emit(nc, scheme, p_pts, chunk, reps):
    f32 = mybir.dt.float32
    bf16 = mybir.dt.bfloat16
    X = mybir.AxisListType.X
    MAX = mybir.AluOpType.max

    if scheme == "rt":
        lhs_d = nc.dram_tensor(
            "lhs", [64, p_pts // 2], bf16, kind="ExternalInput"
        ).ap()
        rhs_d = nc.dram_tensor(
            "rhs", [64, p_pts], bf16, kind="ExternalInput"
        ).ap()
    else:
        lhs_d = nc.dram_tensor(
            "lhs", [K_ROWS, p_pts], bf16, kind="ExternalInput"
        ).ap()
        rhs_d = nc.dram_tensor(
            "rhs", [K_ROWS, p_pts], bf16, kind="ExternalInput"
        ).ap()
    out_d = nc.dram_tensor("out", [128, 1], f32, kind="ExternalOutput").ap()

    nb = p_pts // 128  # p-blocks
    nch = p_pts // chunk  # q-chunks per block
    mm_n = min(512, chunk)  # matmul free-dim width
    mmpc = chunk // mm_n  # matmuls per chunk

    with tile.TileContext(nc) as tc, ExitStack() as ctx:
        inp = ctx.enter_context(tc.tile_pool(name="inp", bufs=1))
        # ttr4 allocates [128, 1024] psum tiles regardless of chunk
        ps_cols = 1024 if scheme == "ttr4" else chunk
        psum_bufs = max(2, 8 // max(1, ps_cols // 512))  # 8 banks, bank-padded tiles
        psump = ctx.enter_context(
            tc.tile_pool(name="psum", bufs=psum_bufs, space=bass.MemorySpace.PSUM)
        )
        deep = {"ttr5": 8, "ttr6": 8, "ttr3b": 2, "ttr3c": 4}.get(scheme, 3)
        stagep = ctx.enter_context(tc.tile_pool(name="stage", bufs=deep))
        junkp = ctx.enter_context(tc.tile_pool(name="junk", bufs=deep))
        accp = ctx.enter_context(tc.tile_pool(name="acc", bufs=6))
        resp = ctx.enter_context(tc.tile_pool(name="res", bufs=1))

        if scheme == "rt":
            lhs_sb = inp.tile([64, p_pts // 2], bf16, tag="lhs")
            rhs_sb = inp.tile([64, p_pts], bf16, tag="rhs")
        else:
            lhs_sb = inp.tile([K_ROWS, p_pts], bf16, tag="lhs")
            rhs_sb = inp.tile([K_ROWS, p_pts], bf16, tag="rhs")
        nc.sync.dma_start(lhs_sb[:], lhs_d[:])
        nc.sync.dma_start(rhs_sb[:], rhs_d[:])

        # reps>1: hardware loop around the body — used only for timing
        # (amortizes host/RPC overhead without growing code size)
        loop_cm = tc.For_i(0, reps, 1) if reps > 1 else nullcontext()
        with loop_cm:
            blockmax = resp.tile([128, nb], f32, tag="blockmax")
            if scheme == "simple":
                chmax = resp.tile([128, nb * nch], f32, tag="chmax")
            elif scheme in ("ttr2", "ttr3", "ttr3b", "ttr3c", "ttr4", "ttr5", "ttr6", "ttr3d", "rt"):
                # unchained: one column per chunk-pair, tree-reduced at end
                chmax = resp.tile([128, nb * (nch // 2)], f32, tag="chmax")

            if scheme == "rt":
                # 2-way PE row-tiling with the exact ttr3@512 cadence:
                # per 1024-col pair: mmA, mmB, stage(A), dve(B, stagedA).
                # Blocks alternate between row-groups 0 and 32 so
                # consecutive blocks' matmuls run concurrently in the PE
                # (independent 32-row subarrays), halving PE duty.
                for i in range(nb):
                    t, G = i % 2, i // 2
                    wt = lhs_sb[32 * t : 32 * t + K_ROWS, G * 128 : (G + 1) * 128]
                    rr = rhs_sb[32 * t : 32 * t + K_ROWS, :]
                    for j in range(0, nch, 2):
                        psA = psump.tile([128, chunk], f32, tag="ps")
                        nc.tensor.matmul(
                            psA[:],
                            wt,
                            rr[:, j * chunk : (j + 1) * chunk],
                            start=True,
                            stop=True,
                        )
                        psB = psump.tile([128, chunk], f32, tag="ps")
                        nc.tensor.matmul(
                            psB[:],
                            wt,
                            rr[:, (j + 1) * chunk : (j + 2) * chunk],
                            start=True,
                            stop=True,
                        )
                        st = stagep.tile([128, chunk], f32, tag="st")
                        nc.scalar.copy(st[:], psA[:])
                        junk = junkp.tile([128, chunk], f32, tag="junk")
                        col = i * (nch // 2) + j // 2
                        nc.vector._custom_dve(
                            TTR_MAX,
                            out=junk[:],
                            in0=psB[:],
                            in1=st[:],
                            s0=NEG_INF,
                            accum_out=chmax[:, col : col + 1],
                        )
            for i in range(0 if scheme != "rt" else nb, nb):
                wt = lhs_sb[:, i * 128 : (i + 1) * 128]

                if scheme == "simple":
                    for j in range(nch):
                        ps = psump.tile([128, chunk], f32, tag="ps")
                        for m in range(mmpc):
                            q0 = j * chunk + m * mm_n
                            nc.tensor.matmul(
                                ps[:, m * mm_n : (m + 1) * mm_n],
                                wt,
                                rhs_sb[:, q0 : q0 + mm_n],
                                start=True,
                                stop=True,
                            )
                        col = i * nch + j
                        nc.vector.reduce_max(
                            chmax[:, col : col + 1], ps[:], axis=X
                        )
                elif scheme == "ttr4":
                    # groups of 2048 cols: psS [128,1024] staged by ONE ACT
                    # copy, psD [128,1024] read live from PSUM by 1024//chunk
                    # DVE ops of FD=chunk
                    n_ops = 1024 // chunk
                    for g in range(p_pts // 2048):
                        psS = psump.tile([128, 1024], f32, tag="ps")
                        for m in range(2):
                            q0 = g * 2048 + m * 512
                            nc.tensor.matmul(
                                psS[:, m * 512 : (m + 1) * 512],
                                wt,
                                rhs_sb[:, q0 : q0 + 512],
                                start=True,
                                stop=True,
                            )
                        psD = psump.tile([128, 1024], f32, tag="ps")
                        for m in range(2):
                            q0 = g * 2048 + 1024 + m * 512
                            nc.tensor.matmul(
                                psD[:, m * 512 : (m + 1) * 512],
                                wt,
                                rhs_sb[:, q0 : q0 + 512],
                                start=True,
                                stop=True,
                            )
                        st = stagep.tile([128, 1024], f32, tag="st")
                        nc.scalar.copy(st[:], psS[:])
                        for h in range(n_ops):
                            junk = junkp.tile([128, chunk], f32, tag="junk")
                            col = i * (nch // 2) + g * n_ops + h
                            nc.vector._custom_dve(
                                TTR_MAX,
                                out=junk[:],
                                in0=psD[:, h * chunk : (h + 1) * chunk],
                                in1=st[:, h * chunk : (h + 1) * chunk],
                                s0=NEG_INF,
                                accum_out=chmax[:, col : col + 1],
                            )
                else:  # "ttr"/"ttr2"/"ttr3": pairs of chunks, ACT stages one
                    acc_ap = None
                    for j in range(0, nch, 2):
                        psA = psump.tile([128, chunk], f32, tag="ps")
                        for m in range(mmpc):
                            q0 = j * chunk + m * mm_n
                            nc.tensor.matmul(
                                psA[:, m * mm_n : (m + 1) * mm_n],
                                wt,
                                rhs_sb[:, q0 : q0 + mm_n],
                                start=True,
                                stop=True,
                            )
                        if scheme == "ttr3d":
                            st = stagep.tile([128, chunk], f32, tag="st")
                            nc.scalar.copy(st[:], psA[:])
                        psB = psump.tile([128, chunk], f32, tag="ps")
                        for m in range(mmpc):
                            q0 = (j + 1) * chunk + m * mm_n
                            nc.tensor.matmul(
                                psB[:, m * mm_n : (m + 1) * mm_n],
                                wt,
                                rhs_sb[:, q0 : q0 + mm_n],
                                start=True,
                                stop=True,
                            )
                        if scheme == "ttr3d":
                            dve_psum, dve_sbuf = psB, st
                        else:
                            st = stagep.tile([128, chunk], f32, tag="st")
                        if scheme in ("ttr3", "ttr3b", "ttr3c", "ttr5", "ttr6"):
                            # stage the FIRST-produced chunk: ACT gets a full
                            # pipeline slot of lead time, DVE reads the later
                            # chunk straight from PSUM
                            nc.scalar.copy(st[:], psA[:])
                            dve_psum, dve_sbuf = psB, st
                        elif scheme != "ttr3d":
                            nc.scalar.copy(st[:], psB[:])
                            dve_psum, dve_sbuf = psA, st
                        junk = junkp.tile([128, chunk], f32, tag="junk")
                        if scheme in ("ttr2", "ttr3", "ttr3b", "ttr3c", "ttr3d", "ttr5", "ttr6"):
                            col = i * (nch // 2) + j // 2
                            accout = chmax[:, col : col + 1]
                            seed = NEG_INF
                        else:
                            last = j + 2 >= nch
                            if last:
                                accout = blockmax[:, i : i + 1]
                            else:
                                accout = accp.tile(
                                    [128, 1], f32, name="acc", tag="acc"
                                )[:]
                            seed = NEG_INF if acc_ap is None else acc_ap
                        if scheme == "ttr6":  # SBUF on rd0, PSUM on rd1
                            dve_in0, dve_in1 = dve_sbuf, dve_psum
                        else:
                            dve_in0, dve_in1 = dve_psum, dve_sbuf
                        nc.vector._custom_dve(
                            TTR_MAX,
                            out=junk[:],
                            in0=dve_in0[:],
                            in1=dve_in1[:],
                            s0=seed,
                            accum_out=accout,
                        )
                        acc_ap = accout

            if scheme == "simple":
                v = chmax[:].rearrange("p (b c) -> p b c", c=nch)
                nc.vector.tensor_reduce(blockmax[:], v, axis=X, op=MAX)
            elif scheme in ("ttr2", "ttr3", "ttr3b", "ttr3c", "ttr4", "ttr5", "ttr6", "ttr3d", "rt"):
                v = chmax[:].rearrange("p (b c) -> p b c", c=nch // 2)
                nc.vector.tensor_reduce(blockmax[:], v, axis=X, op=MAX)

            sums = resp.tile([128, 1], f32, tag="sums")
            nc.vector.reduce_sum(sums[:], blockmax[:], axis=X)
            nc.sync.dma_start(out_d[:], sums[:])


@functools.lru_cache(maxsize=4)
def _build(scheme="ttr", p_pts=P_PTS, chunk=CHUNK, reps=1):
    nc = bacc.Bacc(
        "TRN2", target_bir_lowering=False, debug=False, num_devices=N_CORES
    )
    _emit(nc, scheme, p_pts, chunk, reps)
    nc.compile()
    return nc


# ---------------------------------------------------------------- executor


class _Exec:
    """Cached jitted SPMD executable for a built Bass module (axon/PJRT)."""

    def __init__(self, nc, n_cores=N_CORES):
        bass2jax.install_neuronx_cc_hook()
        self.nc = nc
        self.n_cores = n_cores
        partition_name = (
            nc.partition_id_tensor.name if nc.partition_id_tensor else None
        )
        in_names, out_names, out_avals = [], [], []
        for alloc in nc.m.functions[0].allocations:
            if not isinstance(alloc, mybir.MemoryLocationSet):
                continue
            name = alloc.memorylocations[0].name
            if alloc.kind == "ExternalInput":
                if name != partition_name:
                    in_names.append(name)
            elif alloc.kind == "ExternalOutput":
                out_names.append(name)
                out_avals.append(
                    jax.core.ShapedArray(
                        tuple(alloc.tensor_shape), mybir.dt.np(alloc.dtype)
                    )
                )
        self.in_names = in_names
        self.out_names = out_names
        self.out_avals = out_avals
        n_params = len(in_names)
        all_names = list(in_names + out_names)
        if partition_name is not None:
            all_names.append(partition_name)
        donate = tuple(range(n_params, n_params + len(out_names)))

        def _body(*args):
            operands = list(args)
            if partition_name is not None:
                operands.append(bass2jax.partition_id_tensor())
            return tuple(
                bass2jax._bass_exec_p.bind(
                    *operands,
                    out_avals=tuple(out_avals),
                    in_names=tuple(all_names),
                    out_names=tuple(out_names),
                    lowering_input_output_aliases=(),
                    sim_require_finite=True,
                    sim_require_nnan=True,
                    nc=nc,
                )
            )

        devices = jax.devices()[:n_cores]
        assert len(devices) == n_cores
        mesh = Mesh(np.asarray(devices), ("core",))
        specs = (PartitionSpec("core"),) * (n_params + len(out_names))
        self._fn = jax.jit(
            shard_map(
                _body,
                mesh=mesh,
                in_specs=specs,
                out_specs=(PartitionSpec("core"),) * len(out_names),
                check_rep=False,
            ),
            donate_argnums=donate,
            keep_unused=True,
        )

    def _concat_inputs(self, in_maps):
        return [
            np.concatenate([np.asarray(m[name]) for m in in_maps], axis=0)
            for name in self.in_names
        ]

    def _zeros(self):
        return [
            np.zeros((self.n_cores * a.shape[0], *a.shape[1:]), a.dtype)
            for a in self.out_avals
        ]

    def run(self, in_maps):
        outs = self._fn(*self._concat_inputs(in_maps), *self._zeros())
        return [
            {
                name: np.asarray(outs[i]).reshape(
                    self.n_cores, *self.out_avals[i].shape
                )[c]
                for i, name in enumerate(self.out_names)
            }
            for c in range(self.n_cores)
        ]

    def time(self, in_maps, iters=20, repeats=3):
        """Per-call wall time (s), inputs device-resident, min over repeats."""
        import time as _time

        cin = [jax.device_put(x) for x in self._concat_inputs(in_maps)]
        jax.block_until_ready(cin)
        outs = self._fn(*cin, *self._zeros())  # warm
        jax.block_until_ready(outs)
        best = float("inf")
        for _ in range(repeats):
            t0 = _time.perf_counter()
            last = None
            for _ in range(iters):
                last = self._fn(*cin, *self._zeros())
            jax.block_until_ready(last)
            t1 = _time.perf_counter()
            best = min(best, (t1 - t0) / iters)
        return best


@functools.lru_cache(maxsize=4)
def _get_exec(scheme="ttr", p_pts=P_PTS, chunk=CHUNK, reps=1):
    return _Exec(_build(scheme, p_pts, chunk, reps))


# ------------------------------------------------------------------- kernel


def _rt_layout(lhs, rhs):
    """Rearrange for 2-way PE row-tiling: block i -> row-group 32*(i%2),
    weight column-group i//2; rhs replicated at partition offsets 0/32."""
    P = lhs.shape[1]
    nb = P // 128
    lhs_t = np.zeros((64, P // 2), BF16)
    for i in range(nb):
        t, G = i % 2, i // 2
        lhs_t[32 * t : 32 * t + K_ROWS, 128 * G : 128 * (G + 1)] = lhs[
            :, 128 * i : 128 * (i + 1)
        ]
    rhs_r = np.zeros((64, P), BF16)
    rhs_r[0:K_ROWS] = rhs
    rhs_r[32 : 32 + K_ROWS] = rhs
    return lhs_t, rhs_r


def _make_in_maps(cloud1, cloud2, scheme=None):
    scheme = SCHEME if scheme is None else scheme
    cloud1 = np.asarray(cloud1)
    cloud2 = np.asarray(cloud2)
    n_batch = cloud1.shape[0]
    assert n_batch * 2 == N_CORES
    in_maps, halves = [], []
    for n in range(n_batch):
        for A, B in ((cloud1[n], cloud2[n]), (cloud2[n], cloud1[n])):
            lhs, rhs, sum_half_a2 = _prep_side(A, B)
            if scheme == "rt":
                lhs, rhs = _rt_layout(lhs, rhs)
            in_maps.append({"lhs": lhs, "rhs": rhs})
            halves.append(sum_half_a2)
    return in_maps, halves


def _combine(results, halves, n_batch, p_pts=P_PTS):
    out = np.zeros(n_batch, np.float64)
    for c in range(len(results)):
        S = float(np.asarray(results[c]["out"], np.float64).sum())
        out[c // 2] += 2.0 * (halves[c] - S) / p_pts
    return out.astype(np.float32)


def kernel(cloud1, cloud2):
    in_maps, halves = _make_in_maps(cloud1, cloud2)
    ex = _get_exec(SCHEME, P_PTS, CHUNK, 1)
    results = ex.run(in_maps)
    return _combine(results, halves, np.asarray(cloud1).shape[0])

